# revision 15
# baseline (speedup 1.0000x reference)
"""Capsule-routing kernel for Trainium2, data-parallel over batch (8 cores).

Math: the reference's per-instance routing (unique -> gather -> attention)
is reformulated as a dense masked softmax over the 64x64 cell grid:
  - Q is folded into the 1x1-conv weights host-side, so one fused per-image
    GEMM (21 output channels: 19 score caps + value-scalar + act-logit)
    produces raw attention scores directly,
  - the relative-position encoding's mean term cancels in the softmax and
    reduces to a rank-1 correction computed from per-instance occupancy sums,
  - per-instance dedup of points happens host-side; the device scatters the
    unique cell ids of each instance (4 gpsimd channels per instance) into a
    bitmap, merged by a 4->1 reduction matmul and clamped to 1,
  - all 32 instances reduce in a single accumulated PE matmul against the
    occupancy mask.
X is cast to fp16 host-side (halves HBM traffic and PE stream cycles);
attention weights run in bf16; accumulation stays fp32 in PSUM.
DMA discipline: dma_start is a blocking ~0.6us instruction on the issuing
sequencer, so X moves as 20 big [128,2048] tiles split across the sync and
gpsimd sequencers, and all small tensors ride one host-packed byte blob.
"""
import sys

sys.path.insert(0, "/opt/trn_rl_repo")

import numpy as np

import concourse.bacc as bacc
import concourse.mybir as mybir
from concourse import masks, tile
from concourse.bass_utils import run_bass_kernel_spmd

F32 = mybir.dt.float32
F16 = mybir.dt.float16
BF16 = mybir.dt.bfloat16
I16 = mybir.dt.int16
U8 = mybir.dt.uint8

B = 8
CIN = 1280
NCELL = 4096  # 64x64 feature grid
NCAPS = 19
NI = 32  # instances per image
EPS = 1e-6
NCH = 10  # channel chunks of 128
NCK = 32  # 128-cell chunks
NO = 21  # fused GEMM outputs: 19 scores + vl + alogit
NOP = 24  # padded output width (keeps slices 16B-aligned)

# byte offsets in the packed small-tensor blob [128, PACKB] u8
OFF_W = 0          # fp16 [128, 264]: 10 x [128,24] chunks + [3,24] tail chunk
OFF_WC = 528       # f32  [128, 64]: interleaved (wcorr, 1) per cell chunk
OFF_S4 = 784       # bf16 [128, 32]: 4->1 group merge matrix
OFF_PT = 848       # i16  [128, 256]: deduped scatter idxs, 4 quarters x 64
OFF_ID = 1360      # f32  [24, 24] identity (partitions 0:24)
PACKB = 1456

_CACHE = {}

# Force every activation onto the one table set that covers exp/ln/copy so
# the ACT engine never reloads its function tables mid-kernel.
_ONE_SET = "natural_log_exp_and_others"
_orig_get_tables = None


def _patched_tables(arch):
    full = _orig_get_tables(arch)
    return {
        name: (funcs if name == _ONE_SET else set())
        for name, funcs in full.items()
    }


def _install_act_table_patch():
    global _orig_get_tables
    if _orig_get_tables is None:
        _orig_get_tables = bacc.get_activation_tables
        bacc.get_activation_tables = _patched_tables


def _build_nc(dbg=False, loop_n=1, mode="full"):
    key = ("nc", dbg, loop_n, mode)
    if key in _CACHE:
        return _CACHE[key]

    _install_act_table_patch()
    nc = bacc.Bacc(None, target_bir_lowering=False, debug=False)
    if dbg:
        M1D = nc.dram_tensor("M1D", [NOP, NCELL], F32, kind="ExternalOutput")
        OCTD = nc.dram_tensor("OCTD", [128, NCK * NI], F32, kind="ExternalOutput")
        PS3D = nc.dram_tensor("PS3D", [NI, 40], F32, kind="ExternalOutput")

    X = nc.dram_tensor("X", [CIN, NCELL], F16, kind="ExternalInput")
    PACK = nc.dram_tensor("PACK", [128, PACKB], U8, kind="ExternalInput")
    POS = nc.dram_tensor("POS", [NOP, NCELL], F32, kind="ExternalInput")
    OUT = nc.dram_tensor("OUT", [NI, NCAPS], F32, kind="ExternalOutput")

    with tile.TileContext(nc) as tc:
        with (
            tc.tile_pool(name="const", bufs=1) as cpool,
            tc.tile_pool(name="xp", bufs=20) as xpool,
            tc.tile_pool(name="m1", bufs=1) as m1pool,
            tc.tile_pool(name="small", bufs=1) as spool,
            tc.tile_pool(name="grp", bufs=2) as gpool,
            tc.tile_pool(name="ps1", bufs=4, space="PSUM") as ps1,
            tc.tile_pool(name="pst", bufs=2, space="PSUM") as pstp,
            tc.tile_pool(name="pso", bufs=1, space="PSUM") as pso,
            tc.tile_pool(name="ps3", bufs=1, space="PSUM") as ps3,
        ):
            # ---- packed small tensors: one DMA, first thing on sync ----
            pk = cpool.tile([128, PACKB], U8)
            nc.sync.dma_start(pk[:], PACK[:])
            wsb = pk[:, OFF_W : OFF_W + 528].bitcast(F16)
            wcsb = pk[:, OFF_WC : OFF_WC + 256].bitcast(F32)
            s4sb = pk[:, OFF_S4 : OFF_S4 + 64].bitcast(BF16)
            ptsi = pk[:, OFF_PT : OFF_PT + 512].bitcast(I16)
            id24 = pk[0:NOP, OFF_ID : OFF_ID + 96].bitcast(F32)

            posm = cpool.tile([NOP, NCELL], F32)

            xres = cpool.tile([128, 2048], F16)
            if mode == "compute":
                nc.sync.dma_start(xres[:], X[0:128, 0:2048])

            def _x_dma(jp, k, eng):
                xt = xpool.tile([128, 2048], F16, tag="xt")
                eng.dma_start(
                    xt[:],
                    X[k * 128 : (k + 1) * 128, jp * 2048 : (jp + 1) * 2048],
                )
                return xt

            def body():
                if mode == "dma":
                    for jp in range(2):
                        for k in range(NCH):
                            _x_dma(jp, k, nc.sync)
                    return

                # X tile DMAs, all issued up front, split across the two
                # sequencer streams. gpsimd first does the scatter chain,
                # then feeds the second half's early k-chunks.
                xts = {}
                if mode != "compute":
                    for k in range(NCH):
                        xts[(0, k)] = _x_dma(0, k, nc.sync)
                nc.sync.dma_start(posm[:], POS[:])
                if mode != "compute":
                    for k in range(NCH):
                        xts[(1, k)] = _x_dma(1, k, nc.sync)

                # ---- occupancy: scatter host-deduped indices ----
                ones128 = spool.tile([128, 64], BF16)
                nc.gpsimd.memset(ones128[:], 1.0)
                occ = spool.tile([128, NCELL], BF16)
                for q in range(4):
                    nc.gpsimd.local_scatter(
                        out_ap=occ[:, q * 1024 : (q + 1) * 1024],
                        data_ap=ones128[:],
                        idxs_ap=ptsi[:, q * 64 : (q + 1) * 64],
                        channels=128,
                        num_elems=1024,
                        num_idxs=64,
                    )
                # occ [4*inst+grp, cell] -> occt [cell, inst] (4->1 merge via
                # S4 then clamp to 1).
                occt = cpool.tile([128, NCK * NI], BF16)

                def occ_quarter(q):
                    pso_q = pso.tile([128, 8 * NI], F32, tag="pso")
                    for s in range(8):
                        jj = 8 * q + s
                        nc.tensor.matmul(
                            pso_q[:, s * NI : (s + 1) * NI],
                            occ[:, jj * 128 : (jj + 1) * 128],
                            s4sb[:],
                        )
                    # clamp merged counts to 1 while copying PSUM->SBUF
                    nc.vector.tensor_scalar(
                        occt[:, q * 8 * NI : (q + 1) * 8 * NI],
                        pso_q[:],
                        1.0,
                        None,
                        op0=mybir.AluOpType.min,
                    )

                # at_all: per chunk jj cols [40jj,40jj+40):
                #   0:19 exp(score+aml), 19:38 *vl, 38:40 (wcorr, 1)
                at_all = cpool.tile([128, NCK * 40], BF16)
                atv = at_all[:].rearrange("p (c k) -> p c k", k=40)
                nc.vector.tensor_copy(atv[:, :, 38:40], wcsb[:])

                m1 = m1pool.tile([NOP, NCELL], F32)
                psum3 = ps3.tile([NI, 40], F32)
                for jp in range(2):
                    psums = []
                    for h in range(4):
                        psum_h = ps1.tile([NOP, 512], F32, tag="ps1")
                        psums.append(psum_h)
                    for k in range(NCH):
                        xt = xts[(jp, k)] if mode != "compute" else xres
                        for h in range(4):
                            nc.tensor.matmul(
                                psums[h][:],
                                wsb[:, k * NOP : (k + 1) * NOP],
                                xt[:, h * 512 : (h + 1) * 512],
                                start=(k == 0),
                                stop=(k == NCH - 1),
                            )
                    occ_quarter(2 * jp)
                    occ_quarter(2 * jp + 1)
                    for h in range(4):
                        j = 4 * jp + h
                        nc.vector.tensor_tensor(
                            m1[:, j * 512 : (j + 1) * 512],
                            psums[h][:],
                            posm[:, j * 512 : (j + 1) * 512],
                            op=mybir.AluOpType.add,
                        )
                        # transpose the 4 chunks of this group: [24,128]->[128,24]
                        pst = pstp.tile([128, 4 * NOP], F32, tag="pst")
                        for s in range(4):
                            jj = 4 * j + s
                            nc.tensor.matmul(
                                pst[:, s * NOP : (s + 1) * NOP],
                                m1[:, jj * 128 : (jj + 1) * 128],
                                id24[:],
                                is_transpose=True,
                            )
                        # gather vl / alogit columns, batch aml for 4 chunks
                        pv = pst[:].rearrange("p (s k) -> p s k", k=NOP)
                        vls = gpool.tile([128, 4], F32, tag="vls")
                        av = gpool.tile([128, 4], F32, tag="av")
                        nc.vector.tensor_copy(vls[:], pv[:, :, 19])
                        nc.vector.tensor_copy(av[:], pv[:, :, 20])
                        # aml = ln(sigmoid(z)+eps) = ln(1+eps+eps*e^-z)-ln(1+e^-z)
                        sg = gpool.tile([128, 4], F32, tag="sg")
                        sgw = gpool.tile([128, 4], F32, tag="sgw")
                        nc.scalar.activation(
                            sg[:], av[:],
                            mybir.ActivationFunctionType.Exp, scale=-1.0,
                        )
                        nc.vector.tensor_scalar(
                            sg[:], sg[:], 1.0, None, op0=mybir.AluOpType.add
                        )
                        nc.vector.tensor_scalar(
                            sgw[:], sg[:], EPS, 1.0,
                            op0=mybir.AluOpType.mult, op1=mybir.AluOpType.add,
                        )
                        nc.scalar.activation(
                            sg[:], sg[:], mybir.ActivationFunctionType.Ln
                        )
                        nc.scalar.activation(
                            sgw[:], sgw[:], mybir.ActivationFunctionType.Ln
                        )
                        aml = gpool.tile([128, 4], F32, tag="aml")
                        nc.vector.tensor_tensor(
                            aml[:], sgw[:], sg[:], op=mybir.AluOpType.subtract
                        )
                        for s in range(4):
                            jj = 4 * j + s
                            nc.scalar.activation(
                                atv[:, jj, 0:19],
                                pst[:, s * NOP : s * NOP + 19],
                                mybir.ActivationFunctionType.Exp,
                                bias=aml[:, s : s + 1],
                            )
                            nc.vector.tensor_scalar(
                                atv[:, jj, 19:38],
                                atv[:, jj, 0:19],
                                vls[:, s : s + 1],
                                None,
                                op0=mybir.AluOpType.mult,
                            )
                        for s in range(4):
                            jj = 4 * j + s
                            nc.tensor.matmul(
                                psum3[:],
                                occt[:, jj * NI : (jj + 1) * NI],
                                atv[:, jj, :],
                                start=(jj == 0),
                                stop=(jj == NCK - 1),
                            )

                # ---- finalize: sigmoid(num/den + corr/n) ----
                rsb = spool.tile([NI, 40], F32)
                nc.scalar.copy(rsb[:], psum3[:])
                if dbg:
                    nc.sync.dma_start(M1D[:], m1[:])
                    nc.sync.dma_start(OCTD[:], occt[:])
                    nc.sync.dma_start(PS3D[:], rsb[:])
                t1 = spool.tile([NI, NCAPS], F32)
                t2 = spool.tile([NI, 1], F32)
                rc1 = spool.tile([NI, NCAPS], F32)
                rc2 = spool.tile([NI, 1], F32)
                nc.vector.reciprocal(rc1[:], rsb[:, 0:NCAPS])
                nc.vector.tensor_tensor(
                    t1[:], rsb[:, NCAPS : 2 * NCAPS], rc1[:],
                    op=mybir.AluOpType.mult,
                )
                nc.vector.reciprocal(rc2[:], rsb[:, 39:40])
                nc.vector.tensor_tensor(
                    t2[:], rsb[:, 38:39], rc2[:], op=mybir.AluOpType.mult
                )
                nc.vector.tensor_scalar(
                    t1[:], t1[:], t2[:], None, op0=mybir.AluOpType.add
                )
                # sigmoid(L) = exp(-ln(1+exp(-L))) with only exp/ln
                osb = spool.tile([NI, NCAPS], F32)
                nc.scalar.activation(
                    osb[:], t1[:], mybir.ActivationFunctionType.Exp, scale=-1.0
                )
                nc.vector.tensor_scalar(
                    osb[:], osb[:], 1.0, None, op0=mybir.AluOpType.add
                )
                nc.scalar.activation(
                    osb[:], osb[:], mybir.ActivationFunctionType.Ln
                )
                nc.scalar.activation(
                    osb[:], osb[:], mybir.ActivationFunctionType.Exp, scale=-1.0
                )
                nc.sync.dma_start(OUT[:], osb[:])

            if loop_n == 1:
                body()
            else:
                with tc.For_i(0, loop_n, 1):
                    body()

    nc.compile()
    _CACHE[key] = nc
    return nc


def _fold_weights(Wp, bp, Wa, ba, Q, Wk, bk, Wv, bv, Wl, bl):
    f = lambda t: np.asarray(t, np.float64)
    Wp, bp, Wa, ba, Q, Wk, bk, Wv, bv, Wl, bl = map(
        f, (Wp, bp, Wa, ba, Q, Wk, bk, Wv, bv, Wl, bl)
    )
    wl = Wl[:, 0]
    QT8 = Q.T / 8.0                       # [64,19]
    WK = Wp.T @ Wk[:256]                  # [1280,64]
    wvl_cap = Wv[:256] @ wl               # [256]
    a, b = Wv[256] @ wl, Wv[257] @ wl

    W_all = np.zeros((CIN + 3, NOP), np.float64)
    W_all[:CIN, 0:19] = WK @ QT8
    W_all[:CIN, 19] = Wp.T @ wvl_cap
    W_all[:CIN, 20] = Wa[0]
    W_all[CIN + 0, 0:19] = (Wk[256] / 64.0) @ QT8
    W_all[CIN + 1, 0:19] = (Wk[257] / 64.0) @ QT8
    W_all[CIN + 2, 0:19] = (bp @ Wk[:256] + bk) @ QT8
    W_all[CIN + 0, 19] = a / 64.0
    W_all[CIN + 1, 19] = b / 64.0
    W_all[CIN + 2, 19] = bp @ wvl_cap + bv @ wl
    W_all[CIN + 2, 20] = ba[0]

    c = np.arange(NCELL)
    y64 = (c // 64) / 64.0
    x64 = (c % 64) / 64.0
    wcorr = -(a * y64 + b * x64 - bl[0])
    WC2 = np.empty((128, 2 * NCK), np.float64)
    WC2[:, 0::2] = wcorr.reshape(NCK, 128).T
    WC2[:, 1::2] = 1.0

    # per-cell positional score contribution, added after the X GEMM.
    # W rows CIN+0/1 already carry the /64 coordinate scale, so multiply
    # by the raw cell coordinates here.
    POSm = (
        (c // 64)[None, :] * W_all[CIN + 0][:, None]
        + (c % 64)[None, :] * W_all[CIN + 1][:, None]
        + W_all[CIN + 2][:, None]
    )

    return (
        W_all.astype(np.float16),
        WC2.astype(np.float32),
        POSm.astype(np.float32),
    )


def _make_in_maps(
    feature_output, Wp, bp, Wa, ba, Q, Wk, bk, Wv, bv, Wl, bl, point_lists
):
    import ml_dtypes

    W_all, WC2, POSm = _fold_weights(Wp, bp, Wa, ba, Q, Wk, bk, Wv, bv, Wl, bl)

    S4 = np.zeros((128, NI), np.float32)
    S4[np.arange(128), np.arange(128) // 4] = 1.0
    S4 = S4.astype(ml_dtypes.bfloat16)

    # wsb layout [128, 264] fp16: chunk k<10 at cols 24k from W rows 128k+p;
    # tail chunk at cols 240:264 rows 1280:1283 on partitions 0:3.
    wsb = np.zeros((128, 11 * NOP), np.float16)
    for k in range(NCH):
        wsb[:, k * NOP : (k + 1) * NOP] = W_all[k * 128 : (k + 1) * 128]
    wsb[0:3, 10 * NOP : 11 * NOP] = W_all[CIN : CIN + 3]

    fo = np.asarray(feature_output, np.float32).astype(np.float16)

    # Host-deduped scatter indices (see kernel docstring).
    pl = np.asarray(point_lists).astype(np.int64)  # [B, NI, 2, 256]
    keys = (pl[:, :, 0] // 16) * 64 + (pl[:, :, 1] // 16)  # [B, NI, 256]
    ptsi = np.full((B, 128, 256), -1, np.int16)
    for i in range(B):
        for n in range(NI):
            u = np.unique(keys[i, n])
            for g in range(4):
                seg = u[64 * g : 64 * (g + 1)]
                if seg.size == 0:
                    continue
                q = seg // 1024
                ptsi[i, 4 * n + g, q * 64 + np.arange(seg.size) % 64] = (
                    seg - 1024 * q
                )

    def pack_one(i):
        blob = np.zeros((128, PACKB), np.uint8)
        blob[:, OFF_W : OFF_W + 528] = wsb.view(np.uint8).reshape(128, 528)
        blob[:, OFF_WC : OFF_WC + 256] = (
            np.ascontiguousarray(WC2).view(np.uint8).reshape(128, 256)
        )
        blob[:, OFF_S4 : OFF_S4 + 64] = (
            np.ascontiguousarray(S4).view(np.uint8).reshape(128, 64)
        )
        blob[:, OFF_PT : OFF_PT + 512] = (
            np.ascontiguousarray(ptsi[i]).view(np.uint8).reshape(128, 512)
        )
        id24 = np.eye(NOP, dtype=np.float32)
        blob[0:NOP, OFF_ID : OFF_ID + 96] = id24.view(np.uint8).reshape(NOP, 96)
        return blob

    return [
        {
            "X": np.ascontiguousarray(fo[i].reshape(CIN, NCELL)),
            "PACK": pack_one(i),
            "POS": POSm,
        }
        for i in range(B)
    ]


def kernel(
    feature_output, Wp, bp, Wa, ba, Q, Wk, bk, Wv, bv, Wl, bl, point_lists
):
    nc = _build_nc()
    in_maps = _make_in_maps(
        feature_output, Wp, bp, Wa, ba, Q, Wk, bk, Wv, bv, Wl, bl, point_lists
    )
    res = run_bass_kernel_spmd(nc, in_maps, core_ids=list(range(B)))
    return np.stack([res.results[i]["OUT"] for i in range(B)]).astype(np.float32)


# revision 17
# speedup vs baseline: 1.0152x; 1.0152x over previous
"""Capsule-routing kernel for Trainium2, data-parallel over batch (8 cores).

Math: the reference's per-instance routing (unique -> gather -> attention)
is reformulated as a dense masked softmax over the 64x64 cell grid:
  - Q is folded into the 1x1-conv weights host-side, so one fused per-image
    GEMM (21 output channels: 19 score caps + value-scalar + act-logit)
    produces raw attention scores directly,
  - the relative-position encoding's mean term cancels in the softmax and
    reduces to a rank-1 correction computed from per-instance occupancy sums,
  - per-instance dedup of points happens host-side; the device scatters the
    unique cell ids of each instance (4 gpsimd channels per instance) into a
    bitmap, merged by a 4->1 reduction matmul and clamped to 1,
  - all 32 instances reduce in a single accumulated PE matmul against the
    occupancy mask.
X is cast to fp16 host-side (halves HBM traffic and PE stream cycles);
attention weights run in bf16; accumulation stays fp32 in PSUM.
DMA discipline: dma_start is a blocking ~0.6us instruction on the issuing
sequencer, so X moves as 20 big [128,2048] tiles split across the sync and
gpsimd sequencers, and all small tensors ride one host-packed byte blob.
"""
import sys

sys.path.insert(0, "/opt/trn_rl_repo")

import numpy as np

import concourse.bacc as bacc
import concourse.mybir as mybir
from concourse import masks, tile
from concourse.bass_utils import run_bass_kernel_spmd

F32 = mybir.dt.float32
F16 = mybir.dt.float16
BF16 = mybir.dt.bfloat16
I16 = mybir.dt.int16
U8 = mybir.dt.uint8

B = 8
CIN = 1280
NCELL = 4096  # 64x64 feature grid
NCAPS = 19
NI = 32  # instances per image
EPS = 1e-6
NCH = 10  # channel chunks of 128
NCK = 32  # 128-cell chunks
NO = 21  # fused GEMM outputs: 19 scores + vl + alogit
NOP = 24  # padded output width (keeps slices 16B-aligned)

# byte offsets in the packed small-tensor blob [128, PACKB] u8
OFF_W = 0          # fp16 [128, 264]: 10 x [128,24] chunks + [3,24] tail chunk
OFF_WC = 528       # f32  [128, 64]: interleaved (wcorr, 1) per cell chunk
OFF_S4 = 784       # bf16 [128, 32]: 4->1 group merge matrix
OFF_PT = 848       # i16  [128, 256]: deduped scatter idxs, 4 quarters x 64
OFF_ID = 1360      # f32  [24, 24] identity (partitions 0:24)
PACKB = 1456

_CACHE = {}

# Force every activation onto the one table set that covers exp/ln/copy so
# the ACT engine never reloads its function tables mid-kernel.
_ONE_SET = "natural_log_exp_and_others"
_orig_get_tables = None


def _patched_tables(arch):
    full = _orig_get_tables(arch)
    return {
        name: (funcs if name == _ONE_SET else set())
        for name, funcs in full.items()
    }


def _install_act_table_patch():
    global _orig_get_tables
    if _orig_get_tables is None:
        _orig_get_tables = bacc.get_activation_tables
        bacc.get_activation_tables = _patched_tables


def _build_nc(dbg=False, loop_n=1, mode="full"):
    key = ("nc", dbg, loop_n, mode)
    if key in _CACHE:
        return _CACHE[key]

    _install_act_table_patch()
    nc = bacc.Bacc(None, target_bir_lowering=False, debug=False)
    if dbg:
        M1D = nc.dram_tensor("M1D", [NOP, NCELL], F32, kind="ExternalOutput")
        OCTD = nc.dram_tensor("OCTD", [128, NCK * NI], F32, kind="ExternalOutput")
        PS3D = nc.dram_tensor("PS3D", [NI, 40], F32, kind="ExternalOutput")

    X = nc.dram_tensor("X", [CIN, NCELL], F16, kind="ExternalInput")
    PACK = nc.dram_tensor("PACK", [128, PACKB], U8, kind="ExternalInput")
    POS = nc.dram_tensor("POS", [NOP, NCELL], F32, kind="ExternalInput")
    OUT = nc.dram_tensor("OUT", [NI, NCAPS], F32, kind="ExternalOutput")

    with tile.TileContext(nc) as tc:
        with (
            tc.tile_pool(name="const", bufs=1) as cpool,
            tc.tile_pool(name="xp", bufs=20) as xpool,
            tc.tile_pool(name="m1", bufs=1) as m1pool,
            tc.tile_pool(name="small", bufs=1) as spool,
            tc.tile_pool(name="grp", bufs=2) as gpool,
            tc.tile_pool(name="ps1", bufs=4, space="PSUM") as ps1,
            tc.tile_pool(name="pst", bufs=2, space="PSUM") as pstp,
            tc.tile_pool(name="pso", bufs=1, space="PSUM") as pso,
            tc.tile_pool(name="ps3", bufs=1, space="PSUM") as ps3,
        ):
            # ---- packed small tensors: one DMA on gpsimd so its
            # completion event is not queued behind the X flood ----
            pk = cpool.tile([128, PACKB], U8)
            nc.gpsimd.dma_start(pk[:], PACK[:])
            wsb = pk[:, OFF_W : OFF_W + 528].bitcast(F16)
            wcsb = pk[:, OFF_WC : OFF_WC + 256].bitcast(F32)
            s4sb = pk[:, OFF_S4 : OFF_S4 + 64].bitcast(BF16)
            ptsi = pk[:, OFF_PT : OFF_PT + 512].bitcast(I16)
            id24 = pk[0:NOP, OFF_ID : OFF_ID + 96].bitcast(F32)

            posm = cpool.tile([NOP, NCELL], F32)

            xres = cpool.tile([128, 2048], F16)
            if mode == "compute":
                nc.sync.dma_start(xres[:], X[0:128, 0:2048])

            def _x_dma(jp, k, eng):
                xt = xpool.tile([128, 2048], F16, tag="xt")
                eng.dma_start(
                    xt[:],
                    X[k * 128 : (k + 1) * 128, jp * 2048 : (jp + 1) * 2048],
                )
                return xt

            def body():
                if mode == "dma":
                    for jp in range(2):
                        for k in range(NCH):
                            _x_dma(jp, k, nc.sync)
                    return

                # X tile DMAs, all issued up front, split across the two
                # sequencer streams. gpsimd first does the scatter chain,
                # then feeds the second half's early k-chunks.
                xts = {}
                if mode != "compute":
                    for k in range(NCH):
                        xts[(0, k)] = _x_dma(0, k, nc.sync)
                nc.sync.dma_start(posm[:], POS[:])
                if mode != "compute":
                    for k in range(NCH):
                        xts[(1, k)] = _x_dma(1, k, nc.sync)

                # ---- occupancy: scatter host-deduped indices ----
                ones128 = spool.tile([128, 64], BF16)
                nc.gpsimd.memset(ones128[:], 1.0)
                occ = spool.tile([128, NCELL], BF16)
                for q in range(4):
                    nc.gpsimd.local_scatter(
                        out_ap=occ[:, q * 1024 : (q + 1) * 1024],
                        data_ap=ones128[:],
                        idxs_ap=ptsi[:, q * 64 : (q + 1) * 64],
                        channels=128,
                        num_elems=1024,
                        num_idxs=64,
                    )
                # occ [4*inst+grp, cell] -> occt [cell, inst] (4->1 merge via
                # S4 then clamp to 1).
                occt = cpool.tile([128, NCK * NI], BF16)

                def occ_quarter(q):
                    pso_q = pso.tile([128, 8 * NI], F32, tag="pso")
                    for s in range(8):
                        jj = 8 * q + s
                        nc.tensor.matmul(
                            pso_q[:, s * NI : (s + 1) * NI],
                            occ[:, jj * 128 : (jj + 1) * 128],
                            s4sb[:],
                        )
                    # clamp merged counts to 1 while copying PSUM->SBUF
                    nc.vector.tensor_scalar(
                        occt[:, q * 8 * NI : (q + 1) * 8 * NI],
                        pso_q[:],
                        1.0,
                        None,
                        op0=mybir.AluOpType.min,
                    )

                # at_all: per chunk jj cols [40jj,40jj+40):
                #   0:19 exp(score+aml), 19:38 *vl, 38:40 (wcorr, 1)
                at_all = cpool.tile([128, NCK * 40], BF16)
                atv = at_all[:].rearrange("p (c k) -> p c k", k=40)
                nc.vector.tensor_copy(atv[:, :, 38:40], wcsb[:])

                m1 = m1pool.tile([NOP, NCELL], F32)
                psum3 = ps3.tile([NI, 40], F32)
                for jp in range(4):
                    # two 512-cell groups packed into one PSUM bank at
                    # partition offsets 0 and 32 (PE tile_position cols)
                    psall = ps1.tile([64, 512], F32, tag="ps1")
                    psums = [psall[0:NOP, :], psall[32 : 32 + NOP, :]]
                    for k in range(NCH):
                        if mode != "compute":
                            xt = xts[(jp // 2, k)][
                                :, (jp % 2) * 1024 : (jp % 2 + 1) * 1024
                            ]
                        else:
                            xt = xres[:, 0:1024]
                        for h in range(2):
                            nc.tensor.matmul(
                                psums[h],
                                wsb[:, k * NOP : (k + 1) * NOP],
                                xt[:, h * 512 : (h + 1) * 512],
                                start=(k == 0),
                                stop=(k == NCH - 1),
                                # the two groups live at partition offsets
                                # 0/32 of one bank; start zeroes only the
                                # written partitions, so packing is safe
                                skip_group_check=True,
                            )
                    occ_quarter(jp)
                    for h in range(2):
                        j = 2 * jp + h
                        nc.vector.tensor_tensor(
                            m1[:, j * 512 : (j + 1) * 512],
                            psums[h],
                            posm[:, j * 512 : (j + 1) * 512],
                            op=mybir.AluOpType.add,
                        )
                        # transpose the 4 chunks of this group: [24,128]->[128,24]
                        pst = pstp.tile([128, 4 * NOP], F32, tag="pst")
                        for s in range(4):
                            jj = 4 * j + s
                            nc.tensor.matmul(
                                pst[:, s * NOP : (s + 1) * NOP],
                                m1[:, jj * 128 : (jj + 1) * 128],
                                id24[:],
                                is_transpose=True,
                            )
                        # gather vl / alogit columns, batch aml for 4 chunks
                        pv = pst[:].rearrange("p (s k) -> p s k", k=NOP)
                        vls = gpool.tile([128, 4], F32, tag="vls")
                        av = gpool.tile([128, 4], F32, tag="av")
                        nc.vector.tensor_copy(vls[:], pv[:, :, 19])
                        nc.vector.tensor_copy(av[:], pv[:, :, 20])
                        # aml = ln(sigmoid(z)+eps) = ln(1+eps+eps*e^-z)-ln(1+e^-z)
                        sg = gpool.tile([128, 4], F32, tag="sg")
                        sgw = gpool.tile([128, 4], F32, tag="sgw")
                        nc.scalar.activation(
                            sg[:], av[:],
                            mybir.ActivationFunctionType.Exp, scale=-1.0,
                        )
                        nc.vector.tensor_scalar(
                            sg[:], sg[:], 1.0, None, op0=mybir.AluOpType.add
                        )
                        nc.vector.tensor_scalar(
                            sgw[:], sg[:], EPS, 1.0,
                            op0=mybir.AluOpType.mult, op1=mybir.AluOpType.add,
                        )
                        nc.scalar.activation(
                            sg[:], sg[:], mybir.ActivationFunctionType.Ln
                        )
                        nc.scalar.activation(
                            sgw[:], sgw[:], mybir.ActivationFunctionType.Ln
                        )
                        aml = gpool.tile([128, 4], F32, tag="aml")
                        nc.vector.tensor_tensor(
                            aml[:], sgw[:], sg[:], op=mybir.AluOpType.subtract
                        )
                        for s in range(4):
                            jj = 4 * j + s
                            nc.scalar.activation(
                                atv[:, jj, 0:19],
                                pst[:, s * NOP : s * NOP + 19],
                                mybir.ActivationFunctionType.Exp,
                                bias=aml[:, s : s + 1],
                            )
                            nc.vector.tensor_scalar(
                                atv[:, jj, 19:38],
                                atv[:, jj, 0:19],
                                vls[:, s : s + 1],
                                None,
                                op0=mybir.AluOpType.mult,
                            )
                        for s in range(4):
                            jj = 4 * j + s
                            nc.tensor.matmul(
                                psum3[:],
                                occt[:, jj * NI : (jj + 1) * NI],
                                atv[:, jj, :],
                                start=(jj == 0),
                                stop=(jj == NCK - 1),
                            )

                # ---- finalize: sigmoid(num/den + corr/n) ----
                rsb = spool.tile([NI, 40], F32)
                nc.scalar.copy(rsb[:], psum3[:])
                if dbg:
                    nc.sync.dma_start(M1D[:], m1[:])
                    nc.sync.dma_start(OCTD[:], occt[:])
                    nc.sync.dma_start(PS3D[:], rsb[:])
                t1 = spool.tile([NI, NCAPS], F32)
                t2 = spool.tile([NI, 1], F32)
                rc1 = spool.tile([NI, NCAPS], F32)
                rc2 = spool.tile([NI, 1], F32)
                nc.vector.reciprocal(rc1[:], rsb[:, 0:NCAPS])
                nc.vector.tensor_tensor(
                    t1[:], rsb[:, NCAPS : 2 * NCAPS], rc1[:],
                    op=mybir.AluOpType.mult,
                )
                nc.vector.reciprocal(rc2[:], rsb[:, 39:40])
                nc.vector.tensor_tensor(
                    t2[:], rsb[:, 38:39], rc2[:], op=mybir.AluOpType.mult
                )
                nc.vector.tensor_scalar(
                    t1[:], t1[:], t2[:], None, op0=mybir.AluOpType.add
                )
                # sigmoid(L) = exp(-ln(1+exp(-L))) with only exp/ln
                osb = spool.tile([NI, NCAPS], F32)
                nc.scalar.activation(
                    osb[:], t1[:], mybir.ActivationFunctionType.Exp, scale=-1.0
                )
                nc.vector.tensor_scalar(
                    osb[:], osb[:], 1.0, None, op0=mybir.AluOpType.add
                )
                nc.scalar.activation(
                    osb[:], osb[:], mybir.ActivationFunctionType.Ln
                )
                nc.scalar.activation(
                    osb[:], osb[:], mybir.ActivationFunctionType.Exp, scale=-1.0
                )
                nc.sync.dma_start(OUT[:], osb[:])

            if loop_n == 1:
                body()
            else:
                with tc.For_i(0, loop_n, 1):
                    body()

    nc.compile()
    _CACHE[key] = nc
    return nc


def _fold_weights(Wp, bp, Wa, ba, Q, Wk, bk, Wv, bv, Wl, bl):
    f = lambda t: np.asarray(t, np.float64)
    Wp, bp, Wa, ba, Q, Wk, bk, Wv, bv, Wl, bl = map(
        f, (Wp, bp, Wa, ba, Q, Wk, bk, Wv, bv, Wl, bl)
    )
    wl = Wl[:, 0]
    QT8 = Q.T / 8.0                       # [64,19]
    WK = Wp.T @ Wk[:256]                  # [1280,64]
    wvl_cap = Wv[:256] @ wl               # [256]
    a, b = Wv[256] @ wl, Wv[257] @ wl

    W_all = np.zeros((CIN + 3, NOP), np.float64)
    W_all[:CIN, 0:19] = WK @ QT8
    W_all[:CIN, 19] = Wp.T @ wvl_cap
    W_all[:CIN, 20] = Wa[0]
    W_all[CIN + 0, 0:19] = (Wk[256] / 64.0) @ QT8
    W_all[CIN + 1, 0:19] = (Wk[257] / 64.0) @ QT8
    W_all[CIN + 2, 0:19] = (bp @ Wk[:256] + bk) @ QT8
    W_all[CIN + 0, 19] = a / 64.0
    W_all[CIN + 1, 19] = b / 64.0
    W_all[CIN + 2, 19] = bp @ wvl_cap + bv @ wl
    W_all[CIN + 2, 20] = ba[0]

    c = np.arange(NCELL)
    y64 = (c // 64) / 64.0
    x64 = (c % 64) / 64.0
    wcorr = -(a * y64 + b * x64 - bl[0])
    WC2 = np.empty((128, 2 * NCK), np.float64)
    WC2[:, 0::2] = wcorr.reshape(NCK, 128).T
    WC2[:, 1::2] = 1.0

    # per-cell positional score contribution, added after the X GEMM.
    # W rows CIN+0/1 already carry the /64 coordinate scale, so multiply
    # by the raw cell coordinates here.
    POSm = (
        (c // 64)[None, :] * W_all[CIN + 0][:, None]
        + (c % 64)[None, :] * W_all[CIN + 1][:, None]
        + W_all[CIN + 2][:, None]
    )

    return (
        W_all.astype(np.float16),
        WC2.astype(np.float32),
        POSm.astype(np.float32),
    )


def _make_in_maps(
    feature_output, Wp, bp, Wa, ba, Q, Wk, bk, Wv, bv, Wl, bl, point_lists
):
    import ml_dtypes

    W_all, WC2, POSm = _fold_weights(Wp, bp, Wa, ba, Q, Wk, bk, Wv, bv, Wl, bl)

    S4 = np.zeros((128, NI), np.float32)
    S4[np.arange(128), np.arange(128) // 4] = 1.0
    S4 = S4.astype(ml_dtypes.bfloat16)

    # wsb layout [128, 264] fp16: chunk k<10 at cols 24k from W rows 128k+p;
    # tail chunk at cols 240:264 rows 1280:1283 on partitions 0:3.
    wsb = np.zeros((128, 11 * NOP), np.float16)
    for k in range(NCH):
        wsb[:, k * NOP : (k + 1) * NOP] = W_all[k * 128 : (k + 1) * 128]
    wsb[0:3, 10 * NOP : 11 * NOP] = W_all[CIN : CIN + 3]

    fo = np.asarray(feature_output, np.float32).astype(np.float16)

    # Host-deduped scatter indices (see kernel docstring).
    pl = np.asarray(point_lists).astype(np.int64)  # [B, NI, 2, 256]
    keys = (pl[:, :, 0] // 16) * 64 + (pl[:, :, 1] // 16)  # [B, NI, 256]
    ptsi = np.full((B, 128, 256), -1, np.int16)
    for i in range(B):
        for n in range(NI):
            u = np.unique(keys[i, n])
            for g in range(4):
                seg = u[64 * g : 64 * (g + 1)]
                if seg.size == 0:
                    continue
                q = seg // 1024
                ptsi[i, 4 * n + g, q * 64 + np.arange(seg.size) % 64] = (
                    seg - 1024 * q
                )

    def pack_one(i):
        blob = np.zeros((128, PACKB), np.uint8)
        blob[:, OFF_W : OFF_W + 528] = wsb.view(np.uint8).reshape(128, 528)
        blob[:, OFF_WC : OFF_WC + 256] = (
            np.ascontiguousarray(WC2).view(np.uint8).reshape(128, 256)
        )
        blob[:, OFF_S4 : OFF_S4 + 64] = (
            np.ascontiguousarray(S4).view(np.uint8).reshape(128, 64)
        )
        blob[:, OFF_PT : OFF_PT + 512] = (
            np.ascontiguousarray(ptsi[i]).view(np.uint8).reshape(128, 512)
        )
        id24 = np.eye(NOP, dtype=np.float32)
        blob[0:NOP, OFF_ID : OFF_ID + 96] = id24.view(np.uint8).reshape(NOP, 96)
        return blob

    return [
        {
            "X": np.ascontiguousarray(fo[i].reshape(CIN, NCELL)),
            "PACK": pack_one(i),
            "POS": POSm,
        }
        for i in range(B)
    ]


def kernel(
    feature_output, Wp, bp, Wa, ba, Q, Wk, bk, Wv, bv, Wl, bl, point_lists
):
    nc = _build_nc()
    in_maps = _make_in_maps(
        feature_output, Wp, bp, Wa, ba, Q, Wk, bk, Wv, bv, Wl, bl, point_lists
    )
    res = run_bass_kernel_spmd(nc, in_maps, core_ids=list(range(B)))
    return np.stack([res.results[i]["OUT"] for i in range(B)]).astype(np.float32)
